# revision 23
# baseline (speedup 1.0000x reference)
"""Trainium2 Bass kernel for nn_Net_24275155157688.

Per batch element (64x64 adjacency x):
  tr_p = trace(x^p), s_p = sum(x^p) for p = 2..5
  feats(i,j) = [tr_{i+2}^(j+1)/4096^(i+j+1), s_{i+2}^(j+1)/4096^(i+j+2)]
  out = feats @ W.T                      (W: [2, 32])

Device computes the 8 scalars {tr_2..tr_5, s_2..s_5} per batch; the tiny
[8192, 32] @ [32, 2] feature FC runs on host.

Device math (per batch, 64x64 tiles, two batches packed per 128 partitions):
  T  = x^T                                  (PE transpose, fp32)
  P2 = x@x, P3 = x@P2, P4 = x@P3            (PE matmul chain, fp16)
  v1 = x@(ones/64), v_k = x@v_{k-1}         (chain columns; s_p = 64*sum(v_p))
  tr2 = sum(x o T),  tr3 = sum(T o P2)
  tr4 = sum(T o P3), tr5 = sum(T o P4)      (DVE/GpSimd products, fp16)
  partition reductions via mask-matmuls on PE + DVE free-axis reduce.

Data parallel across 8 NeuronCores: x[8192] -> 8 x [1024].
"""

import sys
import numpy as np

sys.path.insert(0, "/opt/trn_rl_repo")

import concourse.bass as bass
import concourse.bacc as bacc
import concourse.mybir as mybir
from concourse.tile import TileContext

F32 = mybir.dt.float32
F16 = mybir.dt.float16

NCORES = 8
B, N = 8192, 64
BPC = B // NCORES        # batches per core
ROUND = 4                # pairs per round (chain psum slots per round tile)
GROUP = 8                # pairs per product group (= 2 rounds)
SLOT_W = 256             # fp32 cols per chain slot
# chain slot layout (fp32 col offsets inside the 256-col slot)
O_M1, O_M2, O_M3, O_V4, O_V5 = 0, 65, 130, 195, 196
INV64 = 1.0 / 64.0
HALVES = ((0, 64), (64, 128))


def make_consts():
    ident = np.zeros((128, 64), np.float16)
    for p in range(128):
        ident[p, p % 64] = 1.0
    # mega mask: mega[p, c] = 1 iff c == 15 + (p >= 64); slice for quantity q
    # (q=0..7) is mega[:, 15-2q : 31-2q] -> [128, 16] with col (2q + half) hot
    mega = np.zeros((128, 31), np.float16)
    for p in range(128):
        mega[p, 15 + (1 if p >= 64 else 0)] = 1.0
    # mask32: [128, 64], col (0 + half) hot -> lands sums at out rows base+0/1
    mask32 = np.zeros((128, 32), np.float16)
    for p in range(128):
        mask32[p, 1 if p >= 64 else 0] = 1.0
    return ident, mega, mask32


def build_nc(bpc=BPC):
    pairs = bpc // 2
    n_groups = pairs // GROUP
    assert pairs % GROUP == 0

    nc = bacc.Bacc()
    x_d = nc.dram_tensor("x", [bpc, N, N], F32, kind="ExternalInput")
    ident_d = nc.dram_tensor("ident", [128, 64], F16, kind="ExternalInput")
    mega_d = nc.dram_tensor("mega", [128, 31], F16, kind="ExternalInput")
    m32_d = nc.dram_tensor("mask32", [128, 32], F16, kind="ExternalInput")
    tr_d = nc.dram_tensor("tr_out", [16, 8 * n_groups], F32, kind="ExternalOutput")
    v_d = nc.dram_tensor("v_out", [64, 32 * n_groups], F32, kind="ExternalOutput")

    with TileContext(nc) as tc:
        with (
            tc.tile_pool(name="const", bufs=1) as constp,
            tc.tile_pool(name="x32", bufs=5) as x32p,
            tc.tile_pool(name="x16", bufs=4) as x16p,
            tc.tile_pool(name="t16", bufs=3) as t16p,
            tc.tile_pool(name="m1sb", bufs=3) as m1p,
            tc.tile_pool(name="m2sb", bufs=3) as m2p,
            tc.tile_pool(name="m3sb", bufs=3) as m3p,
            tc.tile_pool(name="prod", bufs=4) as prodp,
            tc.tile_pool(name="coll", bufs=1) as collp,
            tc.tile_pool(name="s5sb", bufs=2) as s5p,
            tc.tile_pool(name="chain", bufs=2, space="PSUM") as chp,
            tc.tile_pool(name="tps", bufs=1, space="PSUM") as tpsp,
            tc.tile_pool(name="trps", bufs=1, space="PSUM") as trpsp,
            tc.tile_pool(name="tcol", bufs=1, space="PSUM") as tcolp,
            tc.tile_pool(name="vps", bufs=1, space="PSUM") as vpsp,
        ):
            ident = constp.tile([128, 64], F16)
            nc.sync.dma_start(out=ident[:], in_=ident_d[:])
            mega = constp.tile([128, 31], F16)
            nc.sync.dma_start(out=mega[:], in_=mega_d[:])
            mask32 = constp.tile([128, 32], F16)
            nc.sync.dma_start(out=mask32[:], in_=m32_d[:])

            tr_coll = collp.tile([16, 8 * n_groups], F32)
            v_coll = collp.tile([64, 32 * n_groups], F32)



            def mask_q(q):
                return mega[:, 15 - 2 * q : 31 - 2 * q]

            v_ps = None
            for g in range(n_groups):
                if g % 16 == 0:
                    # v column-sum accumulator bank: 16 group-slots of 32 cols
                    v_ps = vpsp.tile([128, 512], F32)
                # ---- load & cast x for this group (16 batches = 8 pairs) ----
                x32 = x32p.tile([128, GROUP * 64], F32)
                # batch 16g + 2*pr + hf, row r -> partition 64*hf + r,
                # free block pr (batch-pair) x col c
                src = (
                    x_d.rearrange("b r c -> (b r) c")[g * 1024 : (g + 1) * 1024]
                    .rearrange("(pr p) c -> p pr c", pr=8)
                )
                nc.sync.dma_start(
                    out=x32[:].rearrange("p (k c) -> p k c", c=64), in_=src
                )

                x16 = x16p.tile([128, GROUP * 65], F16)
                nc.gpsimd.memset(
                    x16[:].rearrange("p (k c) -> p k c", c=65)[:, :, 64:65],
                    INV64,
                )
                nc.vector.tensor_copy(
                    x16[:].rearrange("p (k c) -> p k c", c=65)[:, :, 0:64],
                    x32[:].rearrange("p (k c) -> p k c", c=64),
                )

                t16 = t16p.tile([128, GROUP * 64], F16)
                m1sb = m1p.tile([128, GROUP * 65], F16)
                m2sb = m2p.tile([128, GROUP * 65], F16)
                m3sb = m3p.tile([128, GROUP * 65], F16)

                for r in range(2):  # rounds within group
                    ch = chp.tile([128, ROUND * SLOT_W], F32)
                    t_ps = tpsp.tile([128, 256], F32)

                    def chs(s, off, w):
                        return ch[:, s * SLOT_W + off : s * SLOT_W + off + w]

                    def chh(s, lo, hi, off, w):
                        return ch[lo:hi, s * SLOT_W + off : s * SLOT_W + off + w]

                    for s in range(ROUND):
                        jg = r * ROUND + s  # pair index within group
                        for lo, hi in HALVES:
                            # T = x^T via regular fp16 matmul (x16^T @ I)
                            nc.tensor.matmul(
                                t_ps[lo:hi, s * 64 : s * 64 + 64],
                                x16[lo:hi, jg * 65 : jg * 65 + 64],
                                ident[lo:hi, :],
                            )
                    # T round-slice -> t16 (cast to fp16)
                    nc.scalar.copy(
                        out=t16[:, r * 256 : r * 256 + 256],
                        in_=t_ps[:],
                    )
                    for s in range(ROUND):
                        jg = r * ROUND + s
                        for lo, hi in HALVES:
                            # MM1: [P2 | v1] = x @ [x | ones/64]
                            nc.tensor.matmul(
                                chh(s, lo, hi, O_M1, 65),
                                t16[lo:hi, jg * 64 : jg * 64 + 64],
                                x16[lo:hi, jg * 65 : jg * 65 + 65],
                            )
                    nc.scalar.copy(
                        out=m1sb[:, r * 260 : r * 260 + 260].rearrange(
                            "p (s c) -> p s c", c=65
                        ),
                        in_=ch[:].rearrange("p (s c) -> p s c", c=SLOT_W)[
                            :, :, O_M1 : O_M1 + 65
                        ],
                    )
                    for s in range(ROUND):
                        jg = r * ROUND + s
                        mc = slice((r * ROUND + s) * 65, (r * ROUND + s) * 65 + 65)
                        for lo, hi in HALVES:
                            # MM2: [P3 | v2] = x @ [P2 | v1]
                            nc.tensor.matmul(
                                chh(s, lo, hi, O_M2, 65),
                                t16[lo:hi, jg * 64 : jg * 64 + 64],
                                m1sb[lo:hi, mc],
                            )
                    nc.scalar.copy(
                        out=m2sb[:, r * 260 : r * 260 + 260].rearrange(
                            "p (s c) -> p s c", c=65
                        ),
                        in_=ch[:].rearrange("p (s c) -> p s c", c=SLOT_W)[
                            :, :, O_M2 : O_M2 + 65
                        ],
                    )
                    for s in range(ROUND):
                        jg = r * ROUND + s
                        mc = slice(jg * 65, jg * 65 + 65)
                        for lo, hi in HALVES:
                            # MM3: [P4 | v3] = x @ [P3 | v2]
                            nc.tensor.matmul(
                                chh(s, lo, hi, O_M3, 65),
                                t16[lo:hi, jg * 64 : jg * 64 + 64],
                                m2sb[lo:hi, mc],
                            )
                    nc.scalar.copy(
                        out=m3sb[:, r * 260 : r * 260 + 260].rearrange(
                            "p (s c) -> p s c", c=65
                        ),
                        in_=ch[:].rearrange("p (s c) -> p s c", c=SLOT_W)[
                            :, :, O_M3 : O_M3 + 65
                        ],
                    )

                # ---- trace products for the full group (fp16 elementwise) ----
                m1m = m1sb[:].rearrange("p (k c) -> p k c", c=65)[:, :, 0:64]
                m2m = m2sb[:].rearrange("p (k c) -> p k c", c=65)[:, :, 0:64]
                m3m = m3sb[:].rearrange("p (k c) -> p k c", c=65)[:, :, 0:64]
                x16m = x16[:].rearrange("p (k c) -> p k c", c=65)[:, :, 0:64]
                prods = []
                for q, (eng, in1) in enumerate(
                    (
                        (nc.vector, x16m),  # tr2: x o T
                        (nc.gpsimd, m1m),   # tr3: T o P2
                        (nc.gpsimd, m2m),   # tr4: T o P3
                        (nc.gpsimd, m3m),   # tr5: T o P4
                    )
                ):
                    pr = prodp.tile([128, GROUP * 64], F16, tag=f"prod{q}")
                    eng.tensor_mul(
                        pr[:].rearrange("p (k c) -> p k c", c=64),
                        t16[:].rearrange("p (k c) -> p k c", c=64),
                        in1,
                    )
                    prods.append(pr)

                # ---- mask-matmul column sums (partition reduction) ----
                tr_ps = trpsp.tile([128, 512], F32)
                for q in range(4):
                    nc.tensor.matmul(
                        tr_ps[0:16, :],
                        mask_q(q),
                        prods[q][:],
                        start=(q == 0),
                        stop=(q == 3),
                    )
                # colsums(P4) -> t_colps rows 32-33; colsums(T) = rowsums(x)
                # -> tr_ps rows 32-33 (both via mask32 with out base 32)
                t_colps = tcolp.tile([128, 512], F32)
                nc.tensor.matmul(t_colps[32:64, :], mask32[:], m3m)
                nc.tensor.matmul(tr_ps[32:64, :], mask32[:], t16[:])
                # s5 partials: colsums(P4) o rowsums(x)  (rows 32-33)
                sbB = s5p.tile([128, 512], F32)
                nc.scalar.copy(out=sbB[32:64], in_=tr_ps[32:64, :])
                nc.vector.tensor_mul(tr_ps[32:64, :], t_colps[32:64, :], sbB[32:64])
                # stage 2: per-batch totals -> trace collect
                nc.vector.tensor_reduce(
                    tr_coll[:, g * 8 : g * 8 + 8],
                    tr_ps[0:16, :].rearrange("p (j c) -> p j c", c=64),
                    axis=mybir.AxisListType.X,
                    op=mybir.AluOpType.add,
                )

                # ---- v column sums ----
                gm = g % 16
                voff = 32 * gm
                m2v = m2sb[:].rearrange("p (k c) -> p k c", c=65)[:, :, 64:65]
                m3v = m3sb[:].rearrange("p (k c) -> p k c", c=65)[:, :, 64:65]
                for i, vcols in enumerate((m2v, m3v)):
                    nc.tensor.matmul(
                        v_ps[0:16, voff + 8 * i : voff + 8 * i + 8],
                        mask_q(4 + i),
                        vcols,
                    )
                # s5 per-batch totals (rows 32-33) -> v_ps slot 2
                nc.vector.tensor_reduce(
                    v_ps[32:64, voff + 16 : voff + 24],
                    tr_ps[32:64, :].rearrange("p (j c) -> p j c", c=64),
                    axis=mybir.AxisListType.X,
                    op=mybir.AluOpType.add,
                )
                # s4 per-batch totals = sum colsums(P4) -> v_ps slot 3
                nc.vector.tensor_reduce(
                    v_ps[32:64, voff + 24 : voff + 32],
                    t_colps[32:64, :].rearrange("p (j c) -> p j c", c=64),
                    axis=mybir.AxisListType.X,
                    op=mybir.AluOpType.add,
                )
                if gm == 15 or g == n_groups - 1:
                    base = (g // 16) * 512
                    w = voff + 32
                    nc.scalar.copy(
                        out=v_coll[:, base : base + w], in_=v_ps[0:64, 0:w]
                    )

            nc.sync.dma_start(out=tr_d[:], in_=tr_coll[:])
            nc.sync.dma_start(out=v_d[:], in_=v_coll[:])

    nc.compile()
    return nc


# ---------------------------------------------------------------------------
# host side
# ---------------------------------------------------------------------------

_NC_CACHE = {}


def _get_nc(bpc):
    if bpc not in _NC_CACHE:
        _NC_CACHE[bpc] = build_nc(bpc)
    return _NC_CACHE[bpc]


def _host_finish(tr_out, v_out, W, bpc):
    """tr_out [16, 8*ng], v_out [16, 32*ng] for one core -> [bpc, 2]."""
    ng = bpc // 16
    numel = float(N * N)
    # batch id of (g, j, h) = 16g + 2j + h
    gg, jj, hh = np.meshgrid(
        np.arange(ng), np.arange(8), np.arange(2), indexing="ij"
    )
    bidx = (16 * gg + 2 * jj + hh).ravel()
    tr = np.empty((4, bpc), np.float64)
    s = np.empty((4, bpc), np.float64)
    for k in range(4):
        # tr_out[2k+h, 8g+j]
        vals = tr_out[2 * k : 2 * k + 2, :].T.reshape(ng, 8, 2)
        tr[k, bidx] = vals.astype(np.float64).ravel()
    for k in (0, 1):
        # s2, s3 from v-mask slots 0, 1 (rows 8+2k), scaled by 1/64
        vals = v_out[8 + 2 * k : 10 + 2 * k, :].T.reshape(ng, 4, 8, 2)[:, k]
        s[k, bidx] = vals.astype(np.float64).ravel() * 64.0
    # s4 = sum(P4) exact: v_out rows 32-33, slot 3
    vals = v_out[32:34, :].T.reshape(ng, 4, 8, 2)[:, 3]
    s[2, bidx] = vals.astype(np.float64).ravel()
    # s5 = colsums(P4)·rowsums(x) exact: v_out rows 32-33, slot 2
    vals = v_out[32:34, :].T.reshape(ng, 4, 8, 2)[:, 2]
    s[3, bidx] = vals.astype(np.float64).ravel()
    feats = np.empty((bpc, 32), np.float64)
    for i in range(4):
        gsc = tr[i] / numel
        hsc = s[i] / numel
        for j in range(4):
            feats[:, 4 * i + j] = gsc ** (j + 1) / numel**i
            feats[:, 16 + 4 * i + j] = hsc ** (j + 1) / numel ** (i + 1)
    return feats @ W.astype(np.float64).T


def _run(x, W, trace=False):
    from concourse.bass_utils import run_bass_kernel_spmd

    ident, mega, mask32 = make_consts()
    nc = _get_nc(BPC)
    in_maps = [
        {
            "x": np.ascontiguousarray(x[c * BPC : (c + 1) * BPC]),
            "ident": ident,
            "mega": mega,
            "mask32": mask32,
        }
        for c in range(NCORES)
    ]
    r = run_bass_kernel_spmd(nc, in_maps, list(range(NCORES)), trace=trace)
    res = r.results
    out = np.empty((B, 2), np.float32)
    for c in range(NCORES):
        out[c * BPC : (c + 1) * BPC] = _host_finish(
            res[c]["tr_out"], res[c]["v_out"], W, BPC
        ).astype(np.float32)
    return out, r


def kernel(x, W):
    return _run(x, W)[0]


def run_traced(x, W):
    out, r = _run(x, W, trace=True)
    return r.exec_time_ns



# revision 24
# speedup vs baseline: 1.0056x; 1.0056x over previous
"""Trainium2 Bass kernel for nn_Net_24275155157688.

Per batch element (64x64 adjacency x):
  tr_p = trace(x^p), s_p = sum(x^p) for p = 2..5
  feats(i,j) = [tr_{i+2}^(j+1)/4096^(i+j+1), s_{i+2}^(j+1)/4096^(i+j+2)]
  out = feats @ W.T                      (W: [2, 32])

Device computes the 8 scalars {tr_2..tr_5, s_2..s_5} per batch; the tiny
[8192, 32] @ [32, 2] feature FC runs on host.

Device math (per batch, 64x64 tiles, two batches packed per 128 partitions):
  T  = x^T                                  (PE transpose, fp32)
  P2 = x@x, P3 = x@P2, P4 = x@P3            (PE matmul chain, fp16)
  v1, v2, v3 ride the chain's 65th column; s2 = 64*sum(v2), s3 = 64*sum(v3)
  s4 = sum(P4); s5 = colsums(P4). rowsums(x) (mask-matmul colsum streams)
  tr2 = sum(x o T),  tr3 = sum(T o P2)
  tr4 = sum(T o P3), tr5 = sum(T o P4)      (DVE/GpSimd products, fp16)
  partition reductions via mask-matmuls on PE + DVE free-axis reduce.

Data parallel across 8 NeuronCores: x[8192] -> 8 x [1024].
"""

import sys
import numpy as np

sys.path.insert(0, "/opt/trn_rl_repo")

import concourse.bass as bass
import concourse.bacc as bacc
import concourse.mybir as mybir
from concourse.tile import TileContext

F32 = mybir.dt.float32
F16 = mybir.dt.float16

NCORES = 8
B, N = 8192, 64
BPC = B // NCORES        # batches per core
ROUND = 4                # pairs per round (chain psum slots per round tile)
GROUP = 8                # pairs per product group (= 2 rounds)
SLOT_W = 256             # fp32 cols per chain slot
# chain slot layout (fp32 col offsets inside the 256-col slot)
O_M1, O_M2, O_M3, O_V4, O_V5 = 0, 65, 130, 195, 196
INV64 = 1.0 / 64.0
HALVES = ((0, 64), (64, 128))


def make_consts():
    ident = np.zeros((128, 64), np.float16)
    for p in range(128):
        ident[p, p % 64] = 1.0
    # mega mask: mega[p, c] = 1 iff c == 15 + (p >= 64); slice for quantity q
    # (q=0..7) is mega[:, 15-2q : 31-2q] -> [128, 16] with col (2q + half) hot
    mega = np.zeros((128, 31), np.float16)
    for p in range(128):
        mega[p, 15 + (1 if p >= 64 else 0)] = 1.0
    # mask32: [128, 64], col (0 + half) hot -> lands sums at out rows base+0/1
    mask32 = np.zeros((128, 32), np.float16)
    for p in range(128):
        mask32[p, 1 if p >= 64 else 0] = 1.0
    return ident, mega, mask32


def build_nc(bpc=BPC):
    pairs = bpc // 2
    n_groups = pairs // GROUP
    assert pairs % GROUP == 0

    nc = bacc.Bacc()
    x_d = nc.dram_tensor("x", [bpc, N, N], F32, kind="ExternalInput")
    ident_d = nc.dram_tensor("ident", [128, 64], F16, kind="ExternalInput")
    mega_d = nc.dram_tensor("mega", [128, 31], F16, kind="ExternalInput")
    m32_d = nc.dram_tensor("mask32", [128, 32], F16, kind="ExternalInput")
    tr_d = nc.dram_tensor("tr_out", [16, 8 * n_groups], F32, kind="ExternalOutput")
    v_d = nc.dram_tensor("v_out", [64, 32 * n_groups], F32, kind="ExternalOutput")

    with TileContext(nc) as tc:
        with (
            tc.tile_pool(name="const", bufs=1) as constp,
            tc.tile_pool(name="x32", bufs=3) as x32p,
            tc.tile_pool(name="x16", bufs=3) as x16p,
            tc.tile_pool(name="t16", bufs=3) as t16p,
            tc.tile_pool(name="m1sb", bufs=3) as m1p,
            tc.tile_pool(name="m2sb", bufs=3) as m2p,
            tc.tile_pool(name="m3sb", bufs=3) as m3p,
            tc.tile_pool(name="prod", bufs=3) as prodp,
            tc.tile_pool(name="coll", bufs=1) as collp,
            tc.tile_pool(name="s5sb", bufs=2) as s5p,
            tc.tile_pool(name="chain", bufs=2, space="PSUM") as chp,
            tc.tile_pool(name="tps", bufs=1, space="PSUM") as tpsp,
            tc.tile_pool(name="trps", bufs=1, space="PSUM") as trpsp,
            tc.tile_pool(name="tcol", bufs=1, space="PSUM") as tcolp,
            tc.tile_pool(name="vps", bufs=1, space="PSUM") as vpsp,
        ):
            ident = constp.tile([128, 64], F16)
            nc.sync.dma_start(out=ident[:], in_=ident_d[:])
            mega = constp.tile([128, 31], F16)
            nc.sync.dma_start(out=mega[:], in_=mega_d[:])
            mask32 = constp.tile([128, 32], F16)
            nc.sync.dma_start(out=mask32[:], in_=m32_d[:])

            tr_coll = collp.tile([16, 8 * n_groups], F32)
            v_coll = collp.tile([64, 32 * n_groups], F32)



            def mask_q(q):
                return mega[:, 15 - 2 * q : 31 - 2 * q]

            v_ps = None
            for g in range(n_groups):
                if g % 16 == 0:
                    # v column-sum accumulator bank: 16 group-slots of 32 cols
                    v_ps = vpsp.tile([128, 512], F32)
                # ---- load & cast x for this group (16 batches = 8 pairs) ----
                x32 = x32p.tile([128, GROUP * 64], F32)
                # batch 16g + 2*pr + hf, row r -> partition 64*hf + r,
                # free block pr (batch-pair) x col c
                src = (
                    x_d.rearrange("b r c -> (b r) c")[g * 1024 : (g + 1) * 1024]
                    .rearrange("(pr p) c -> p pr c", pr=8)
                )
                nc.sync.dma_start(
                    out=x32[:].rearrange("p (k c) -> p k c", c=64), in_=src
                )

                x16 = x16p.tile([128, GROUP * 65], F16)
                nc.gpsimd.memset(
                    x16[:].rearrange("p (k c) -> p k c", c=65)[:, :, 64:65],
                    INV64,
                )
                nc.vector.tensor_copy(
                    x16[:].rearrange("p (k c) -> p k c", c=65)[:, :, 0:64],
                    x32[:].rearrange("p (k c) -> p k c", c=64),
                )

                t16 = t16p.tile([128, GROUP * 64], F16)
                m1sb = m1p.tile([128, GROUP * 65], F16)
                m2sb = m2p.tile([128, GROUP * 65], F16)
                m3sb = m3p.tile([128, GROUP * 65], F16)

                for r in range(2):  # rounds within group
                    ch = chp.tile([128, ROUND * SLOT_W], F32)
                    t_ps = tpsp.tile([128, 256], F32)

                    def chs(s, off, w):
                        return ch[:, s * SLOT_W + off : s * SLOT_W + off + w]

                    def chh(s, lo, hi, off, w):
                        return ch[lo:hi, s * SLOT_W + off : s * SLOT_W + off + w]

                    for s in range(ROUND):
                        jg = r * ROUND + s  # pair index within group
                        for lo, hi in HALVES:
                            # T = x^T via regular fp16 matmul (x16^T @ I)
                            nc.tensor.matmul(
                                t_ps[lo:hi, s * 64 : s * 64 + 64],
                                x16[lo:hi, jg * 65 : jg * 65 + 64],
                                ident[lo:hi, :],
                            )
                    # T round-slice -> t16 (cast to fp16)
                    nc.scalar.copy(
                        out=t16[:, r * 256 : r * 256 + 256],
                        in_=t_ps[:],
                    )
                    for s in range(ROUND):
                        jg = r * ROUND + s
                        for lo, hi in HALVES:
                            # MM1: [P2 | v1] = x @ [x | ones/64]
                            nc.tensor.matmul(
                                chh(s, lo, hi, O_M1, 65),
                                t16[lo:hi, jg * 64 : jg * 64 + 64],
                                x16[lo:hi, jg * 65 : jg * 65 + 65],
                            )
                    nc.scalar.copy(
                        out=m1sb[:, r * 260 : r * 260 + 260].rearrange(
                            "p (s c) -> p s c", c=65
                        ),
                        in_=ch[:].rearrange("p (s c) -> p s c", c=SLOT_W)[
                            :, :, O_M1 : O_M1 + 65
                        ],
                    )
                    for s in range(ROUND):
                        jg = r * ROUND + s
                        mc = slice((r * ROUND + s) * 65, (r * ROUND + s) * 65 + 65)
                        for lo, hi in HALVES:
                            # MM2: [P3 | v2] = x @ [P2 | v1]
                            nc.tensor.matmul(
                                chh(s, lo, hi, O_M2, 65),
                                t16[lo:hi, jg * 64 : jg * 64 + 64],
                                m1sb[lo:hi, mc],
                            )
                    nc.scalar.copy(
                        out=m2sb[:, r * 260 : r * 260 + 260].rearrange(
                            "p (s c) -> p s c", c=65
                        ),
                        in_=ch[:].rearrange("p (s c) -> p s c", c=SLOT_W)[
                            :, :, O_M2 : O_M2 + 65
                        ],
                    )
                    for s in range(ROUND):
                        jg = r * ROUND + s
                        mc = slice(jg * 65, jg * 65 + 65)
                        for lo, hi in HALVES:
                            # MM3: [P4 | v3] = x @ [P3 | v2]
                            nc.tensor.matmul(
                                chh(s, lo, hi, O_M3, 65),
                                t16[lo:hi, jg * 64 : jg * 64 + 64],
                                m2sb[lo:hi, mc],
                            )
                    nc.scalar.copy(
                        out=m3sb[:, r * 260 : r * 260 + 260].rearrange(
                            "p (s c) -> p s c", c=65
                        ),
                        in_=ch[:].rearrange("p (s c) -> p s c", c=SLOT_W)[
                            :, :, O_M3 : O_M3 + 65
                        ],
                    )

                # ---- trace products for the full group (fp16 elementwise) ----
                m1m = m1sb[:].rearrange("p (k c) -> p k c", c=65)[:, :, 0:64]
                m2m = m2sb[:].rearrange("p (k c) -> p k c", c=65)[:, :, 0:64]
                m3m = m3sb[:].rearrange("p (k c) -> p k c", c=65)[:, :, 0:64]
                x16m = x16[:].rearrange("p (k c) -> p k c", c=65)[:, :, 0:64]
                prods = []
                for q, (eng, in1) in enumerate(
                    (
                        (nc.vector, x16m),  # tr2: x o T
                        (nc.gpsimd, m1m),   # tr3: T o P2
                        (nc.gpsimd, m2m),   # tr4: T o P3
                        (nc.gpsimd, m3m),   # tr5: T o P4
                    )
                ):
                    pr = prodp.tile([128, GROUP * 64], F16, tag=f"prod{q}")
                    eng.tensor_mul(
                        pr[:].rearrange("p (k c) -> p k c", c=64),
                        t16[:].rearrange("p (k c) -> p k c", c=64),
                        in1,
                    )
                    prods.append(pr)

                # ---- mask-matmul column sums (partition reduction) ----
                tr_ps = trpsp.tile([128, 512], F32)
                for q in range(4):
                    nc.tensor.matmul(
                        tr_ps[0:16, :],
                        mask_q(q),
                        prods[q][:],
                        start=(q == 0),
                        stop=(q == 3),
                    )
                # colsums(P4) -> t_colps rows 32-33; colsums(T) = rowsums(x)
                # -> tr_ps rows 32-33 (both via mask32 with out base 32)
                t_colps = tcolp.tile([128, 512], F32)
                nc.tensor.matmul(t_colps[32:64, :], mask32[:], m3m)
                nc.tensor.matmul(tr_ps[32:64, :], mask32[:], t16[:])
                # s5 partials: colsums(P4) o rowsums(x)  (rows 32-33)
                sbB = s5p.tile([128, 512], F32)
                nc.scalar.copy(out=sbB[32:64], in_=tr_ps[32:64, :])
                nc.vector.tensor_mul(tr_ps[32:64, :], t_colps[32:64, :], sbB[32:64])
                # stage 2: per-batch totals -> trace collect
                nc.vector.tensor_reduce(
                    tr_coll[:, g * 8 : g * 8 + 8],
                    tr_ps[0:16, :].rearrange("p (j c) -> p j c", c=64),
                    axis=mybir.AxisListType.X,
                    op=mybir.AluOpType.add,
                )

                # ---- v column sums ----
                gm = g % 16
                voff = 32 * gm
                m2v = m2sb[:].rearrange("p (k c) -> p k c", c=65)[:, :, 64:65]
                m3v = m3sb[:].rearrange("p (k c) -> p k c", c=65)[:, :, 64:65]
                for i, vcols in enumerate((m2v, m3v)):
                    nc.tensor.matmul(
                        v_ps[0:16, voff + 8 * i : voff + 8 * i + 8],
                        mask_q(4 + i),
                        vcols,
                    )
                # s5 per-batch totals (rows 32-33) -> v_ps slot 2
                nc.vector.tensor_reduce(
                    v_ps[32:64, voff + 16 : voff + 24],
                    tr_ps[32:64, :].rearrange("p (j c) -> p j c", c=64),
                    axis=mybir.AxisListType.X,
                    op=mybir.AluOpType.add,
                )
                # s4 per-batch totals = sum colsums(P4) -> v_ps slot 3
                nc.vector.tensor_reduce(
                    v_ps[32:64, voff + 24 : voff + 32],
                    t_colps[32:64, :].rearrange("p (j c) -> p j c", c=64),
                    axis=mybir.AxisListType.X,
                    op=mybir.AluOpType.add,
                )
                if gm == 15 or g == n_groups - 1:
                    base = (g // 16) * 512
                    w = voff + 32
                    nc.scalar.copy(
                        out=v_coll[:, base : base + w], in_=v_ps[0:64, 0:w]
                    )

            nc.sync.dma_start(out=tr_d[:], in_=tr_coll[:])
            nc.sync.dma_start(out=v_d[:], in_=v_coll[:])

    nc.compile()
    return nc


# ---------------------------------------------------------------------------
# host side
# ---------------------------------------------------------------------------

_NC_CACHE = {}


def _get_nc(bpc):
    if bpc not in _NC_CACHE:
        _NC_CACHE[bpc] = build_nc(bpc)
    return _NC_CACHE[bpc]


def _host_finish(tr_out, v_out, W, bpc):
    """tr_out [16, 8*ng], v_out [16, 32*ng] for one core -> [bpc, 2]."""
    ng = bpc // 16
    numel = float(N * N)
    # batch id of (g, j, h) = 16g + 2j + h
    gg, jj, hh = np.meshgrid(
        np.arange(ng), np.arange(8), np.arange(2), indexing="ij"
    )
    bidx = (16 * gg + 2 * jj + hh).ravel()
    tr = np.empty((4, bpc), np.float64)
    s = np.empty((4, bpc), np.float64)
    for k in range(4):
        # tr_out[2k+h, 8g+j]
        vals = tr_out[2 * k : 2 * k + 2, :].T.reshape(ng, 8, 2)
        tr[k, bidx] = vals.astype(np.float64).ravel()
    for k in (0, 1):
        # s2, s3 from v-mask slots 0, 1 (rows 8+2k), scaled by 1/64
        vals = v_out[8 + 2 * k : 10 + 2 * k, :].T.reshape(ng, 4, 8, 2)[:, k]
        s[k, bidx] = vals.astype(np.float64).ravel() * 64.0
    # s4 = sum(P4) exact: v_out rows 32-33, slot 3
    vals = v_out[32:34, :].T.reshape(ng, 4, 8, 2)[:, 3]
    s[2, bidx] = vals.astype(np.float64).ravel()
    # s5 = colsums(P4)·rowsums(x) exact: v_out rows 32-33, slot 2
    vals = v_out[32:34, :].T.reshape(ng, 4, 8, 2)[:, 2]
    s[3, bidx] = vals.astype(np.float64).ravel()
    feats = np.empty((bpc, 32), np.float64)
    for i in range(4):
        gsc = tr[i] / numel
        hsc = s[i] / numel
        for j in range(4):
            feats[:, 4 * i + j] = gsc ** (j + 1) / numel**i
            feats[:, 16 + 4 * i + j] = hsc ** (j + 1) / numel ** (i + 1)
    return feats @ W.astype(np.float64).T


def _run(x, W, trace=False):
    from concourse.bass_utils import run_bass_kernel_spmd

    ident, mega, mask32 = make_consts()
    nc = _get_nc(BPC)
    in_maps = [
        {
            "x": np.ascontiguousarray(x[c * BPC : (c + 1) * BPC]),
            "ident": ident,
            "mega": mega,
            "mask32": mask32,
        }
        for c in range(NCORES)
    ]
    r = run_bass_kernel_spmd(nc, in_maps, list(range(NCORES)), trace=trace)
    res = r.results
    out = np.empty((B, 2), np.float32)
    for c in range(NCORES):
        out[c * BPC : (c + 1) * BPC] = _host_finish(
            res[c]["tr_out"], res[c]["v_out"], W, BPC
        ).astype(np.float32)
    return out, r


def kernel(x, W):
    return _run(x, W)[0]


def run_traced(x, W):
    out, r = _run(x, W, trace=True)
    return r.exec_time_ns

